# revision 38
# baseline (speedup 1.0000x reference)
"""Trainium2 Bass kernel for a custom transformer block.

Sharding: 8 cores = 4 batches x 2 sequence halves. Each core computes the
full block (LN1 -> QKV -> windowed attention -> LN2 -> MLP -> residual) for
its 1024 query tokens; the KV window (last 1024 tokens of its batch) is
recomputed on both cores of a batch pair to avoid any collectives.

All matmuls run in bf16 (fp32r costs ~2.2x bf16 per row on HW despite the
cost model claiming parity). Layernorm stats use bn_stats; the pad mask is
folded into the exp bias (per-partition = per-key), and the causal mask is
applied post-exp with tensor_paged_mask using per-core index tables, so no
mask tensor is DMA'd. The MLP accumulates over all of D_FF in PSUM and the
final evacuation fuses +b2 and the residual add in one DVE pass.
"""
import sys
import os

if "/opt/trn_rl_repo" not in sys.path:
    sys.path.insert(0, "/opt/trn_rl_repo")

import numpy as np
import ml_dtypes

B, S, D = 4, 2048, 1024
N_HEAD = 16
D_HEAD = 64
WINDOW = 1024
D_FF = 4096
EPS = 1e-5
ISD = float(1.0 / np.sqrt(D))  # 1/32
P = 128
TINY = 1e-30  # rowsum guard: fully-masked rows -> attn 0 instead of inf/NaN

_CACHE = {}


def _build_program():
    import concourse.bacc as bacc
    import concourse.mybir as mybir
    from concourse.tile import TileContext
    from concourse.masks import make_identity

    F32 = mybir.dt.float32
    BF16 = mybir.dt.bfloat16
    F16 = mybir.dt.float16
    AF = mybir.ActivationFunctionType
    ALU = mybir.AluOpType

    nc = bacc.Bacc("TRN2", target_bir_lowering=False, debug=False,
                   num_devices=8)

    xin_d = nc.dram_tensor("xin", [2 * WINDOW, D], BF16, kind="ExternalInput")
    wq_d = nc.dram_tensor("wq", [D, D], BF16, kind="ExternalInput")
    wkv_d = nc.dram_tensor("wkv", [D, 2 * D], BF16, kind="ExternalInput")
    w1_d = nc.dram_tensor("w1", [D, D_FF], BF16, kind="ExternalInput")
    w2_d = nc.dram_tensor("w2", [D_FF, D], BF16, kind="ExternalInput")
    bqs_d = nc.dram_tensor("bqs", [P, 8], F32, kind="ExternalInput")
    bkvk_d = nc.dram_tensor("bkvk", [P, 8], F32, kind="ExternalInput")
    bkvvb_d = nc.dram_tensor("bkvvb", [P, D], F32, kind="ExternalInput")
    g1dm_d = nc.dram_tensor("g1dm", [P, 8], F32, kind="ExternalInput")
    b1dm_d = nc.dram_tensor("b1dm", [P, 8], F32, kind="ExternalInput")
    g2dm_d = nc.dram_tensor("g2dm", [P, 8], F32, kind="ExternalInput")
    b2dm_d = nc.dram_tensor("b2dm", [P, 8], F32, kind="ExternalInput")
    b1s_d = nc.dram_tensor("b1s", [P, 32], F32, kind="ExternalInput")
    b2s_d = nc.dram_tensor("b2s", [P, 8], F32, kind="ExternalInput")
    biasp_d = nc.dram_tensor("biasp", [P, 8], F32, kind="ExternalInput")
    mask01_d = nc.dram_tensor("mask01", [P, 8, WINDOW], BF16,
                              kind="ExternalInput")
    xinT_d = nc.dram_tensor("xinT", [D, WINDOW], F32, kind="ExternalInput")
    y_d = nc.dram_tensor("y", [D, WINDOW], F32, kind="ExternalOutput")

    with TileContext(nc) as tc:
        cpool = tc.alloc_tile_pool(name="const", bufs=1, side="left")
        ident = cpool.tile([P, P], BF16)
        make_identity(nc, ident[:])
        smallc = cpool.tile([P, 96], F32)
        bqs = smallc[:, 0:8]
        bkvk = smallc[:, 8:16]
        b1s = smallc[:, 16:48]
        b2s = smallc[:, 48:56]
        g1dm = smallc[:, 56:64]
        b1dm = smallc[:, 64:72]
        g2dm = smallc[:, 72:80]
        b2dm = smallc[:, 80:88]
        biasp = smallc[:, 88:96]
        onesc = cpool.tile([P, 16], BF16)
        nc.vector.memset(onesc, 1.0)
        nc.sync.dma_start(bqs, bqs_d[:])
        nc.sync.dma_start(bkvk, bkvk_d[:])
        nc.sync.dma_start(b1s, b1s_d[:])
        nc.sync.dma_start(b2s, b2s_d[:])
        nc.sync.dma_start(g1dm, g1dm_d[:])
        nc.sync.dma_start(b1dm, b1dm_d[:])
        nc.sync.dma_start(g2dm, g2dm_d[:])
        nc.sync.dma_start(b2dm, b2dm_d[:])
        nc.sync.dma_start(biasp, biasp_d[:])

        attnp = tc.alloc_tile_pool(name="attn", bufs=1, side="left")
        attn = attnp.tile([P, 8, D], BF16)         # normalized attn out

        # ---------------- Phase B: LN1 + transpose to dim-major ------------
        zTp = tc.alloc_tile_pool(name="zT", bufs=1, side="left")
        zqT = zTp.tile([P, 8, WINDOW], BF16)
        zwT = zTp.tile([P, 8, WINDOW], BF16)
        xz = tc.alloc_tile_pool(name="xz", bufs=3, side="left")
        psC = tc.alloc_tile_pool(name="psC", bufs=2, space="PSUM")
        psB = tc.alloc_tile_pool(name="psB", bufs=3, space="PSUM")

        def ln1_tile(t):
            xt = xz.tile([P, D], BF16, tag="x")
            nc.sync.dma_start(xt[:], xin_d[t * P:(t + 1) * P, :])
            st = xz.tile([P, 20], F32, tag="stats")
            st6 = st[:, 0:12].rearrange("p (j k) -> p j k", k=6)
            mv = st[:, 12:14]
            veps, sdv, rstd = st[:, 14:15], st[:, 15:16], st[:, 16:17]
            nc.vector.bn_stats(st6[:, 0, :], xt[:, 0:512])
            nc.vector.bn_stats(st6[:, 1, :], xt[:, 512:1024])
            nc.vector.bn_aggr(mv, st[:, 0:12])
            nc.vector.tensor_scalar(veps, mv[:, 1:2], 1.0, EPS,
                                    op0=ALU.mult, op1=ALU.add)
            nc.scalar.sqrt(sdv, veps)
            nc.vector.reciprocal(rstd, sdv)
            z = xz.tile([P, D], BF16, tag="z")
            nc.vector.tensor_scalar(z[:], xt[:], mv[:, 0:1], rstd,
                                    op0=ALU.subtract, op1=ALU.mult)
            dst = zqT if t < 8 else zwT
            col = (t % 8) * P
            for c in range(8):
                tp = psB.tile([P, P], BF16, tag="tpB")
                nc.tensor.transpose(tp[:], z[:, c * P:(c + 1) * P], ident[:])
                nc.vector.tensor_scalar(dst[:, c, col:col + P], tp[:],
                                        g1dm[:, c:c + 1], b1dm[:, c:c + 1],
                                        op0=ALU.mult, op1=ALU.add)

        for t in range(8):
            ln1_tile(t)

        qkvp = tc.alloc_tile_pool(name="qkv", bufs=1, side="right")
        qT = qkvp.tile([P, 8, WINDOW], BF16)      # q/sqrt(D), dim-major
        kT = qkvp.tile([P, 8, WINDOW], BF16)      # k, dim-major
        V = qkvp.tile([P, 8, N_HEAD * 65], BF16)  # token-major + ones col

        wst = tc.alloc_tile_pool(name="wst", bufs=2, side="left")

        # Q: weights stationary -> qT dim-major, scaled by 1/32
        for wh in range(2):
            wqr = wst.tile([P, 8, 512], BF16, tag="wkres")
            for kc in range(8):
                nc.sync.dma_start(
                    wqr[:, kc, :],
                    wq_d[kc * P:(kc + 1) * P, wh * 512:(wh + 1) * 512])
            for co in range(wh * 4, wh * 4 + 4):
                for qh in range(2):
                    pp = psC.tile([P, 512], F32, tag="proj")
                    for kc in range(8):
                        nc.tensor.matmul(
                            pp[:], wqr[:, kc, (co % 4) * P:(co % 4 + 1) * P],
                            zqT[:, kc, qh * 512:(qh + 1) * 512],
                            start=(kc == 0), stop=(kc == 7))
                    nc.scalar.activation(
                        qT[:, co, qh * 512:(qh + 1) * 512], pp[:],
                        AF.Identity, bias=bqs[:, co:co + 1], scale=ISD)

        for t in range(8, 16):
            ln1_tile(t)
        psB.release()

        # V: activations stationary -> token-major, bias added via bcast tile
        bkvvb = wst.tile([P, D], F32, tag="bkvvb")
        nc.sync.dma_start(bkvvb[:], bkvvb_d[:])
        for vh in range(2):
            wvr = wst.tile([P, 8, 512], BF16, tag="wkres")
            for kc in range(8):
                nc.sync.dma_start(
                    wvr[:, kc, :],
                    wkv_d[kc * P:(kc + 1) * P,
                          D + vh * 512:D + (vh + 1) * 512])
            for tt in range(8):
                pp = psC.tile([P, 512], F32, tag="proj")
                for kc in range(8):
                    nc.tensor.matmul(
                        pp[:], zwT[:, kc, tt * P:(tt + 1) * P],
                        wvr[:, kc, :],
                        start=(kc == 0), stop=(kc == 7))
                vdst = V[:, tt, :].rearrange("p (h n) -> p h n", n=65)[
                    :, vh * 8:(vh + 1) * 8, 0:64]
                nc.vector.scalar_tensor_tensor(
                    vdst, pp[:].rearrange("p (h n) -> p h n", n=64), 0.0,
                    bkvvb[:, vh * 512:(vh + 1) * 512].rearrange(
                        "p (h n) -> p h n", n=64),
                    op0=ALU.add, op1=ALU.add)
        for tt in range(8):
            nc.scalar.copy(
                V[:, tt, :].rearrange("p (h n) -> p h n", n=65)[:, :, 64:65],
                onesc.rearrange("p (h n) -> p h n", n=1))

        # ---------------- Phase D: K proj interleaved with attention -------
        # K chunk co covers dims [co*128, (co+1)*128) = heads 2*co, 2*co+1,
        # so attention head-pair co can start right after K chunk co.
        mkp = tc.alloc_tile_pool(name="mk", bufs=1, side="left")
        mask01 = mkp.tile([P, 8, WINDOW], BF16)
        nc.sync.dma_start(mask01[:], mask01_d[:])
        sbD = tc.alloc_tile_pool(name="sbD", bufs=2, side="left")
        ptp = tc.alloc_tile_pool(name="ptp", bufs=17, side="right")
        rsp = tc.alloc_tile_pool(name="rsp", bufs=4, side="left")
        psDs = tc.alloc_tile_pool(name="psDs", bufs=4, space="PSUM")
        psDa = tc.alloc_tile_pool(name="psDa", bufs=1, space="PSUM")
        psDt = tc.alloc_tile_pool(name="psDt", bufs=1, space="PSUM")

        wkk = [None, None]

        def kproj_chunk(co):
            wh, cq = co // 4, co % 4
            if cq == 0:
                wkr = wst.tile([P, 8, 512], BF16, tag="wkres")
                wkk[wh] = wkr
                for kc in range(8):
                    nc.sync.dma_start(
                        wkr[:, kc, :],
                        wkv_d[kc * P:(kc + 1) * P, wh * 512:(wh + 1) * 512])
            wkr = wkk[wh]
            for qh in range(2):
                pp = psC.tile([P, 512], F32, tag="proj")
                for kc in range(8):
                    nc.tensor.matmul(
                        pp[:], wkr[:, kc, cq * P:(cq + 1) * P],
                        zwT[:, kc, qh * 512:(qh + 1) * 512],
                        start=(kc == 0), stop=(kc == 7))
                nc.scalar.activation(
                    kT[:, co, qh * 512:(qh + 1) * 512], pp[:],
                    AF.Identity, bias=bkvk[:, co:co + 1], scale=1.0)

        def attn_pair(hp):
            pair = (2 * hp, 2 * hp + 1)
            # scores + exp + causal mask; consecutive matmuls alternate PE
            # row groups (partitions 0-63 vs 64-127) so LDWEIGHTS pulls ahead
            pts = {h: [] for h in pair}
            for kc in range(8):
                pss = {}
                for qh in range(2):
                    for h in pair:
                        po, ch = (h % 2) * 64, h // 2
                        stile = psDs.tile([P, 512], F32, tag="s")
                        pss[(h, qh)] = stile
                        nc.tensor.matmul(
                            stile[:],
                            kT[po:po + 64, ch, kc * P:(kc + 1) * P],
                            qT[po:po + 64, ch, qh * 512:(qh + 1) * 512],
                            start=True, stop=True)
                for h in pair:
                    ptile = ptp.tile([P, 1024], BF16, tag="pt")
                    pts[h].append(ptile)
                    # exp with pad mask folded into the per-key bias
                    for qh in range(2):
                        nc.scalar.activation(
                            ptile[:, qh * 512:(qh + 1) * 512],
                            pss[(h, qh)][:], AF.Exp,
                            bias=biasp[:, kc:kc + 1])
                    # causal mask: multiply probs by 0/1 (kc<4, qh=1 is
                    # never causally masked on any core -> skip that half)
                    ncol = 1024 if kc >= 4 else 512
                    nc.vector.tensor_tensor(
                        ptile[:, 0:ncol], ptile[:, 0:ncol],
                        mask01[:, kc, 0:ncol], op=ALU.mult)
            for h in pair:
                oa = sbD.tile([65, 1024], BF16, tag="oa")
                for qh in range(2):
                    avp = psDa.tile([65, 512], F32, tag="av")
                    for kc in range(8):
                        nc.tensor.matmul(
                            avp[:], V[:, kc, h * 65:(h + 1) * 65],
                            pts[h][kc][:, qh * 512:(qh + 1) * 512],
                            start=(kc == 0), stop=(kc == 7))
                    nc.scalar.copy(oa[:, qh * 512:(qh + 1) * 512], avp[:])
                for tq in range(2):
                    tp4 = psDt.tile([P, 4, 66], BF16, tag="tp4")
                    for j in range(4):
                        t = tq * 4 + j
                        nc.tensor.transpose(tp4[:, j, 0:65],
                                            oa[:, t * P:(t + 1) * P],
                                            ident[0:65, 0:65])
                    rs = rsp.tile([P, 8], F32, tag="rs")
                    rs4, rinv4 = rs[:, 0:4], rs[:, 4:8]
                    nc.vector.tensor_scalar(rs4, tp4[:, :, 64:65], TINY, None,
                                            op0=ALU.add)
                    nc.vector.reciprocal(rinv4, rs4)
                    for j in range(4):
                        t = tq * 4 + j
                        nc.vector.tensor_scalar_mul(
                            attn[:, t, h * 64:(h + 1) * 64],
                            tp4[:, j, 0:64], rinv4[:, j:j + 1])

        for co in range(8):
            kproj_chunk(co)
            attn_pair(co)

        psDt.release()
        psDa.release()
        psDs.release()
        rsp.release()
        ptp.release()
        sbD.release()
        mkp.release()
        qkvp.release()
        psC.release()
        wst.release()
        xz.release()
        zTp.release()

        # ---------------- Phase E: LN2 + transpose ---------------------------
        wf1 = tc.alloc_tile_pool(name="wf1", bufs=2, side="right")
        wf2 = tc.alloc_tile_pool(name="wf2", bufs=3, side="right")
        z2Tp = tc.alloc_tile_pool(name="z2T", bufs=1, side="right")
        z2T = z2Tp.tile([P, 8, WINDOW], BF16)
        xz2 = tc.alloc_tile_pool(name="xz2", bufs=3, side="left")
        psE = tc.alloc_tile_pool(name="psE", bufs=3, space="PSUM")

        for t in range(8):
            at = attn[:, t, :]
            st = xz2.tile([P, 20], F32, tag="stats2")
            st6 = st[:, 0:12].rearrange("p (j k) -> p j k", k=6)
            mv = st[:, 12:14]
            veps, sdv, rstd = st[:, 14:15], st[:, 15:16], st[:, 16:17]
            nc.vector.bn_stats(st6[:, 0, :], at[:, 0:512])
            nc.vector.bn_stats(st6[:, 1, :], at[:, 512:1024])
            nc.vector.bn_aggr(mv, st[:, 0:12])
            nc.vector.tensor_scalar(veps, mv[:, 1:2], 1.0, EPS,
                                    op0=ALU.mult, op1=ALU.add)
            nc.scalar.sqrt(sdv, veps)
            nc.vector.reciprocal(rstd, sdv)
            z = xz2.tile([P, D], BF16, tag="zE")
            nc.vector.tensor_scalar(z[:], at, mv[:, 0:1], rstd,
                                    op0=ALU.subtract, op1=ALU.mult)
            for c in range(8):
                tp = psE.tile([P, P], BF16, tag="tpE")
                nc.tensor.transpose(tp[:], z[:, c * P:(c + 1) * P], ident[:])
                nc.vector.tensor_scalar(z2T[:, c, t * P:(t + 1) * P], tp[:],
                                        g2dm[:, c:c + 1], b2dm[:, c:c + 1],
                                        op0=ALU.mult, op1=ALU.add)

        psE.release()
        xz2.release()

        # ---------------- Phase F: MLP ---------------------------------------
        # h1 for all of D_FF is computed first (bf16, 64KB/partition), then h2
        # accumulates all 32 contraction chunks in PSUM per (co, qh); the
        # evacuation fuses +b2 and the residual in a single DVE pass.
        xinp = tc.alloc_tile_pool(name="xinp", bufs=4, side="left")
        h1p = tc.alloc_tile_pool(name="h1p", bufs=1, side="left")
        h1all = h1p.tile([P, 32, WINDOW], BF16)
        psF1 = tc.alloc_tile_pool(name="psF1", bufs=4, space="PSUM")
        psF2 = tc.alloc_tile_pool(name="psF2", bufs=4, space="PSUM")
        yp = tc.alloc_tile_pool(name="yp", bufs=2, side="left")

        for sc in range(4):
            w1r = wf1.tile([P, 8, 1024], BF16, tag="w1r")
            for kc in range(8):
                nc.sync.dma_start(
                    w1r[:, kc, :],
                    w1_d[kc * P:(kc + 1) * P, sc * 1024:(sc + 1) * 1024])
            for ft in range(8):
                for qh in range(2):
                    hp = psF1.tile([P, 512], F32, tag="h1ps")
                    for kc in range(8):
                        nc.tensor.matmul(
                            hp[:], w1r[:, kc, ft * P:(ft + 1) * P],
                            z2T[:, kc, qh * 512:(qh + 1) * 512],
                            start=(kc == 0), stop=(kc == 7))
                    nc.scalar.activation(
                        h1all[:, sc * 8 + ft, qh * 512:(qh + 1) * 512],
                        hp[:], AF.Silu,
                        bias=b1s[:, sc * 8 + ft:sc * 8 + ft + 1], scale=1.0)
        z2Tp.release()

        for co in range(8):
            w2r = wf2.tile([P, 32, P], BF16, tag="w2r")
            for fc in range(32):
                nc.sync.dma_start(
                    w2r[:, fc, :],
                    w2_d[fc * P:(fc + 1) * P, co * P:(co + 1) * P])
            xt4 = xinp.tile([P, WINDOW], F32, tag="xinT")
            nc.sync.dma_start(xt4[:], xinT_d[co * P:(co + 1) * P, :])
            y = yp.tile([P, WINDOW], F32, tag="y")
            for qh in range(2):
                hp2 = psF2.tile([P, 512], F32, tag="h2ps")
                for fc in range(32):
                    nc.tensor.matmul(
                        hp2[:], w2r[:, fc, :],
                        h1all[:, fc, qh * 512:(qh + 1) * 512],
                        start=(fc == 0), stop=(fc == 31))
                nc.vector.scalar_tensor_tensor(
                    y[:, qh * 512:(qh + 1) * 512], hp2[:],
                    b2s[:, co:co + 1],
                    xt4[:, qh * 512:(qh + 1) * 512],
                    op0=ALU.add, op1=ALU.add)
            nc.sync.dma_start(y_d[co * P:(co + 1) * P, :], y[:])

        yp.release()
        psF2.release()
        psF1.release()
        h1p.release()
        xinp.release()
        wf2.release()
        wf1.release()
        attnp.release()
        cpool.release()

    nc.compile()
    return nc


def _prep_inputs(inputs):
    x = np.asarray(inputs["x"], dtype=np.float32)
    kpm = np.asarray(inputs["key_pad_mask"]).astype(bool)
    wq = np.asarray(inputs["wq"], dtype=np.float32)
    wkv = np.asarray(inputs["wkv"], dtype=np.float32)
    w1 = np.asarray(inputs["w1"], dtype=np.float32)
    w2 = np.asarray(inputs["w2"], dtype=np.float32)
    bq = np.asarray(inputs["bq"], dtype=np.float32)
    bkv = np.asarray(inputs["bkv"], dtype=np.float32)
    b1 = np.asarray(inputs["b1"], dtype=np.float32)
    b2 = np.asarray(inputs["b2"], dtype=np.float32)
    ln1_g = np.asarray(inputs["ln1_g"], dtype=np.float32)
    ln1_b = np.asarray(inputs["ln1_b"], dtype=np.float32)
    ln2_g = np.asarray(inputs["ln2_g"], dtype=np.float32)
    ln2_b = np.asarray(inputs["ln2_b"], dtype=np.float32)

    def dm(v):  # [D] -> [P, 8] dim-major chunk layout
        return np.ascontiguousarray(v.reshape(8, P).T)

    def bf(a):
        return np.ascontiguousarray(a.astype(ml_dtypes.bfloat16))

    shared = {
        "wq": bf(wq),
        "wkv": bf(wkv),
        "w1": bf(w1),
        "w2": bf(w2),
        "bqs": np.ascontiguousarray((bq * ISD).reshape(8, P).T),
        "bkvk": dm(bkv[0:D]),
        "bkvvb": np.ascontiguousarray(
            np.broadcast_to(bkv[D:2 * D], (P, D)).astype(np.float32)),
        "g1dm": dm(ln1_g),
        "b1dm": dm(ln1_b),
        "g2dm": dm(ln2_g),
        "b2dm": dm(ln2_b),
        "b1s": np.ascontiguousarray(b1.reshape(32, P).T),
        "b2s": dm(b2),
    }

    kwin = np.arange(WINDOW, dtype=np.int32)[:, None]  # window key index
    qloc = np.arange(WINDOW, dtype=np.int32)[None, :]  # local query index
    in_maps = []
    for core in range(8):
        b, h = core // 2, core % 2
        xq = x[b, h * WINDOW:(h + 1) * WINDOW]
        xw = x[b, S - WINDOW:S]
        pad = kpm[b, S - WINDOW:S]  # [WINDOW] per-key pad mask
        biasp = np.where(pad.reshape(8, P).T, np.float32(-160.0),
                         np.float32(0.0))
        keep = (kwin <= h * WINDOW + qloc)  # [WINDOW keys, WINDOW queries]
        mask01 = keep.reshape(8, P, WINDOW).transpose(1, 0, 2)
        m = dict(shared)
        m["xin"] = bf(np.concatenate([xq, xw], axis=0))
        m["xinT"] = np.ascontiguousarray(xq.T)
        m["biasp"] = np.ascontiguousarray(biasp)
        m["mask01"] = bf(mask01.astype(np.float32))
        in_maps.append(m)
    return in_maps


def kernel(**inputs):
    from concourse.bass_utils import run_bass_kernel_spmd

    if "nc" not in _CACHE:
        _CACHE["nc"] = _build_program()
    nc = _CACHE["nc"]

    in_maps = _prep_inputs(inputs)
    trace = os.environ.get("KERNEL_TRACE", "0") == "1"
    res = run_bass_kernel_spmd(nc, in_maps, core_ids=list(range(8)),
                               trace=trace)
    if res.exec_time_ns is not None:
        print(f"HW exec time: {res.exec_time_ns} ns")
        _CACHE["exec_time_ns"] = res.exec_time_ns
    out = np.empty((B, S, D), dtype=np.float32)
    for core in range(8):
        b, h = core // 2, core % 2
        out[b, h * WINDOW:(h + 1) * WINDOW] = res.results[core]["y"].T
    return out


# revision 42
# speedup vs baseline: 1.0529x; 1.0529x over previous
"""Trainium2 Bass kernel for a custom transformer block.

Sharding: 8 cores = 4 batches x 2 sequence halves. Each core computes the
full block (LN1 -> QKV -> windowed attention -> LN2 -> MLP -> residual) for
its 1024 query tokens; the KV window (last 1024 tokens of its batch) is
recomputed on both cores of a batch pair to avoid any collectives.

All matmuls run in bf16 (fp32r costs ~2.2x bf16 per row on HW despite the
cost model claiming parity). Layernorm stats use bn_stats; the pad mask is
folded into the exp bias (per-partition = per-key), and the causal mask is
applied post-exp with tensor_paged_mask using per-core index tables, so no
mask tensor is DMA'd. The MLP accumulates over all of D_FF in PSUM and the
final evacuation fuses +b2 and the residual add in one DVE pass.
"""
import sys
import os

if "/opt/trn_rl_repo" not in sys.path:
    sys.path.insert(0, "/opt/trn_rl_repo")

import numpy as np
import ml_dtypes

B, S, D = 4, 2048, 1024
N_HEAD = 16
D_HEAD = 64
WINDOW = 1024
D_FF = 4096
EPS = 1e-5
ISD = float(1.0 / np.sqrt(D))  # 1/32
P = 128
TINY = 1e-30  # rowsum guard: fully-masked rows -> attn 0 instead of inf/NaN

_CACHE = {}


def _build_program():
    import concourse.bacc as bacc
    import concourse.mybir as mybir
    from concourse.tile import TileContext
    from concourse.masks import make_identity

    F32 = mybir.dt.float32
    BF16 = mybir.dt.bfloat16
    F16 = mybir.dt.float16
    AF = mybir.ActivationFunctionType
    ALU = mybir.AluOpType

    nc = bacc.Bacc("TRN2", target_bir_lowering=False, debug=False,
                   num_devices=8)

    xin_d = nc.dram_tensor("xin", [2 * WINDOW, D], BF16, kind="ExternalInput")
    wq_d = nc.dram_tensor("wq", [D, D], BF16, kind="ExternalInput")
    wkv_d = nc.dram_tensor("wkv", [D, 2 * D], BF16, kind="ExternalInput")
    w1_d = nc.dram_tensor("w1", [D, D_FF], BF16, kind="ExternalInput")
    w2_d = nc.dram_tensor("w2", [D_FF, D], BF16, kind="ExternalInput")
    bqs_d = nc.dram_tensor("bqs", [P, 8], F32, kind="ExternalInput")
    bkvk_d = nc.dram_tensor("bkvk", [P, 8], F32, kind="ExternalInput")
    bkvvb_d = nc.dram_tensor("bkvvb", [P, D], F32, kind="ExternalInput")
    g1dm_d = nc.dram_tensor("g1dm", [P, 8], F32, kind="ExternalInput")
    b1dm_d = nc.dram_tensor("b1dm", [P, 8], F32, kind="ExternalInput")
    g2dm_d = nc.dram_tensor("g2dm", [P, 8], F32, kind="ExternalInput")
    b2dm_d = nc.dram_tensor("b2dm", [P, 8], F32, kind="ExternalInput")
    b1s_d = nc.dram_tensor("b1s", [P, 32], F32, kind="ExternalInput")
    b2s_d = nc.dram_tensor("b2s", [P, 8], F32, kind="ExternalInput")
    biasp_d = nc.dram_tensor("biasp", [P, 8], F32, kind="ExternalInput")
    mask01_d = nc.dram_tensor("mask01", [P, 8, WINDOW], BF16,
                              kind="ExternalInput")
    xinT_d = nc.dram_tensor("xinT", [D, WINDOW], F32, kind="ExternalInput")
    y_d = nc.dram_tensor("y", [D, WINDOW], F32, kind="ExternalOutput")

    with TileContext(nc) as tc:
        cpool = tc.alloc_tile_pool(name="const", bufs=1, side="left")
        ident = cpool.tile([P, P], BF16)
        make_identity(nc, ident[:])
        smallc = cpool.tile([P, 96], F32)
        bqs = smallc[:, 0:8]
        bkvk = smallc[:, 8:16]
        b1s = smallc[:, 16:48]
        b2s = smallc[:, 48:56]
        g1dm = smallc[:, 56:64]
        b1dm = smallc[:, 64:72]
        g2dm = smallc[:, 72:80]
        b2dm = smallc[:, 80:88]
        biasp = smallc[:, 88:96]
        onesc = cpool.tile([P, 16], BF16)
        nc.vector.memset(onesc, 1.0)
        nc.sync.dma_start(bqs, bqs_d[:])
        nc.sync.dma_start(bkvk, bkvk_d[:])
        nc.sync.dma_start(b1s, b1s_d[:])
        nc.sync.dma_start(b2s, b2s_d[:])
        nc.sync.dma_start(g1dm, g1dm_d[:])
        nc.sync.dma_start(b1dm, b1dm_d[:])
        nc.sync.dma_start(g2dm, g2dm_d[:])
        nc.sync.dma_start(b2dm, b2dm_d[:])
        nc.sync.dma_start(biasp, biasp_d[:])

        attnp = tc.alloc_tile_pool(name="attn", bufs=1, side="left")
        attn = attnp.tile([P, 8, D], BF16)         # normalized attn out

        # ---------------- Phase B: LN1 + transpose to dim-major ------------
        zTp = tc.alloc_tile_pool(name="zT", bufs=1, side="left")
        zqT = zTp.tile([P, 8, WINDOW], BF16)
        zwT = zTp.tile([P, 8, WINDOW], BF16)
        xz = tc.alloc_tile_pool(name="xz", bufs=3, side="left")
        psC = tc.alloc_tile_pool(name="psC", bufs=2, space="PSUM")
        psB = tc.alloc_tile_pool(name="psB", bufs=3, space="PSUM")

        def ln1_tile(t):
            xt = xz.tile([P, D], BF16, tag="x")
            nc.sync.dma_start(xt[:], xin_d[t * P:(t + 1) * P, :])
            st = xz.tile([P, 20], F32, tag="stats")
            st6 = st[:, 0:12].rearrange("p (j k) -> p j k", k=6)
            mv = st[:, 12:14]
            veps, sdv, rstd = st[:, 14:15], st[:, 15:16], st[:, 16:17]
            nc.vector.bn_stats(st6[:, 0, :], xt[:, 0:512])
            nc.vector.bn_stats(st6[:, 1, :], xt[:, 512:1024])
            nc.vector.bn_aggr(mv, st[:, 0:12])
            nc.vector.tensor_scalar(veps, mv[:, 1:2], 1.0, EPS,
                                    op0=ALU.mult, op1=ALU.add)
            nc.scalar.sqrt(sdv, veps)
            nc.vector.reciprocal(rstd, sdv)
            z = xz.tile([P, D], BF16, tag="z")
            nc.vector.tensor_scalar(z[:], xt[:], mv[:, 0:1], rstd,
                                    op0=ALU.subtract, op1=ALU.mult)
            dst = zqT if t < 8 else zwT
            col = (t % 8) * P
            for c in range(8):
                tp = psB.tile([P, P], BF16, tag="tpB")
                nc.tensor.transpose(tp[:], z[:, c * P:(c + 1) * P], ident[:])
                nc.scalar.activation(dst[:, c, col:col + P], tp[:],
                                     AF.Identity, bias=b1dm[:, c:c + 1],
                                     scale=g1dm[:, c:c + 1])

        for t in range(8):
            ln1_tile(t)

        qkvp = tc.alloc_tile_pool(name="qkv", bufs=1, side="right")
        qT = qkvp.tile([P, 8, WINDOW], BF16)      # q/sqrt(D), dim-major
        kT = qkvp.tile([P, 8, WINDOW], BF16)      # k, dim-major
        V = qkvp.tile([P, 8, N_HEAD * 65], BF16)  # token-major + ones col

        wst = tc.alloc_tile_pool(name="wst", bufs=2, side="left")

        # Q: weights stationary -> qT dim-major, scaled by 1/32
        for wh in range(2):
            wqr = wst.tile([P, 8, 512], BF16, tag="wkres")
            for kc in range(8):
                nc.sync.dma_start(
                    wqr[:, kc, :],
                    wq_d[kc * P:(kc + 1) * P, wh * 512:(wh + 1) * 512])
            for co in range(wh * 4, wh * 4 + 4):
                for qh in range(2):
                    pp = psC.tile([P, 512], F32, tag="proj")
                    for kc in range(8):
                        nc.tensor.matmul(
                            pp[:], wqr[:, kc, (co % 4) * P:(co % 4 + 1) * P],
                            zqT[:, kc, qh * 512:(qh + 1) * 512],
                            start=(kc == 0), stop=(kc == 7))
                    nc.scalar.activation(
                        qT[:, co, qh * 512:(qh + 1) * 512], pp[:],
                        AF.Identity, bias=bqs[:, co:co + 1], scale=ISD)

        for t in range(8, 16):
            ln1_tile(t)
        psB.release()

        # V: activations stationary -> token-major, bias added via bcast tile
        bkvvb = wst.tile([P, D], F32, tag="bkvvb")
        nc.sync.dma_start(bkvvb[:], bkvvb_d[:])
        for vh in range(2):
            wvr = wst.tile([P, 8, 512], BF16, tag="wkres")
            for kc in range(8):
                nc.sync.dma_start(
                    wvr[:, kc, :],
                    wkv_d[kc * P:(kc + 1) * P,
                          D + vh * 512:D + (vh + 1) * 512])
            for tt in range(8):
                pp = psC.tile([P, 512], F32, tag="proj")
                for kc in range(8):
                    nc.tensor.matmul(
                        pp[:], zwT[:, kc, tt * P:(tt + 1) * P],
                        wvr[:, kc, :],
                        start=(kc == 0), stop=(kc == 7))
                vdst = V[:, tt, :].rearrange("p (h n) -> p h n", n=65)[
                    :, vh * 8:(vh + 1) * 8, 0:64]
                nc.vector.scalar_tensor_tensor(
                    vdst, pp[:].rearrange("p (h n) -> p h n", n=64), 0.0,
                    bkvvb[:, vh * 512:(vh + 1) * 512].rearrange(
                        "p (h n) -> p h n", n=64),
                    op0=ALU.add, op1=ALU.add)
        for tt in range(8):
            nc.scalar.copy(
                V[:, tt, :].rearrange("p (h n) -> p h n", n=65)[:, :, 64:65],
                onesc.rearrange("p (h n) -> p h n", n=1))

        # ---------------- Phase D: K proj interleaved with attention -------
        # K chunk co covers dims [co*128, (co+1)*128) = heads 2*co, 2*co+1,
        # so attention head-pair co can start right after K chunk co.
        mkp = tc.alloc_tile_pool(name="mk", bufs=1, side="left")
        mask01 = mkp.tile([P, 8, WINDOW], BF16)
        nc.sync.dma_start(mask01[:], mask01_d[:])
        sbD = tc.alloc_tile_pool(name="sbD", bufs=2, side="left")
        ptp = tc.alloc_tile_pool(name="ptp", bufs=17, side="right")
        rsp = tc.alloc_tile_pool(name="rsp", bufs=4, side="left")
        psDs = tc.alloc_tile_pool(name="psDs", bufs=4, space="PSUM")
        psDa = tc.alloc_tile_pool(name="psDa", bufs=1, space="PSUM")
        psDt = tc.alloc_tile_pool(name="psDt", bufs=1, space="PSUM")

        wkk = [None, None]

        def kproj_chunk(co):
            wh, cq = co // 4, co % 4
            if cq == 0:
                wkr = wst.tile([P, 8, 512], BF16, tag="wkres")
                wkk[wh] = wkr
                for kc in range(8):
                    nc.sync.dma_start(
                        wkr[:, kc, :],
                        wkv_d[kc * P:(kc + 1) * P, wh * 512:(wh + 1) * 512])
            wkr = wkk[wh]
            for qh in range(2):
                pp = psC.tile([P, 512], F32, tag="proj")
                for kc in range(8):
                    nc.tensor.matmul(
                        pp[:], wkr[:, kc, cq * P:(cq + 1) * P],
                        zwT[:, kc, qh * 512:(qh + 1) * 512],
                        start=(kc == 0), stop=(kc == 7))
                nc.vector.tensor_scalar(
                    kT[:, co, qh * 512:(qh + 1) * 512], pp[:],
                    bkvk[:, co:co + 1], None, op0=ALU.add)

        def attn_pair(hp):
            pair = (2 * hp, 2 * hp + 1)
            # scores + exp + causal mask; consecutive matmuls alternate PE
            # row groups (partitions 0-63 vs 64-127) so LDWEIGHTS pulls ahead
            pts = {h: [] for h in pair}
            for kc in range(8):
                pss = {}
                for qh in range(2):
                    for h in pair:
                        po, ch = (h % 2) * 64, h // 2
                        stile = psDs.tile([P, 512], F32, tag="s")
                        pss[(h, qh)] = stile
                        nc.tensor.matmul(
                            stile[:],
                            kT[po:po + 64, ch, kc * P:(kc + 1) * P],
                            qT[po:po + 64, ch, qh * 512:(qh + 1) * 512],
                            start=True, stop=True)
                for h in pair:
                    ptile = ptp.tile([P, 1024], BF16, tag="pt")
                    pts[h].append(ptile)
                    # exp with pad mask folded into the per-key bias
                    for qh in range(2):
                        nc.scalar.activation(
                            ptile[:, qh * 512:(qh + 1) * 512],
                            pss[(h, qh)][:], AF.Exp,
                            bias=biasp[:, kc:kc + 1])
                    # causal mask: multiply probs by 0/1 (kc<4, qh=1 is
                    # never causally masked on any core -> skip that half)
                    ncol = 1024 if kc >= 4 else 512
                    nc.vector.tensor_tensor(
                        ptile[:, 0:ncol], ptile[:, 0:ncol],
                        mask01[:, kc, 0:ncol], op=ALU.mult)
            for h in pair:
                oa = sbD.tile([65, 1024], BF16, tag="oa")
                for qh in range(2):
                    avp = psDa.tile([65, 512], F32, tag="av")
                    for kc in range(8):
                        nc.tensor.matmul(
                            avp[:], V[:, kc, h * 65:(h + 1) * 65],
                            pts[h][kc][:, qh * 512:(qh + 1) * 512],
                            start=(kc == 0), stop=(kc == 7))
                    nc.vector.tensor_copy(oa[:, qh * 512:(qh + 1) * 512],
                                          avp[:])
                for tq in range(2):
                    tp4 = psDt.tile([P, 4, 66], BF16, tag="tp4")
                    for j in range(4):
                        t = tq * 4 + j
                        nc.tensor.transpose(tp4[:, j, 0:65],
                                            oa[:, t * P:(t + 1) * P],
                                            ident[0:65, 0:65])
                    rs = rsp.tile([P, 8], F32, tag="rs")
                    rs4, rinv4 = rs[:, 0:4], rs[:, 4:8]
                    nc.vector.tensor_scalar(rs4, tp4[:, :, 64:65], TINY, None,
                                            op0=ALU.add)
                    nc.vector.reciprocal(rinv4, rs4)
                    for j in range(4):
                        t = tq * 4 + j
                        nc.vector.tensor_scalar_mul(
                            attn[:, t, h * 64:(h + 1) * 64],
                            tp4[:, j, 0:64], rinv4[:, j:j + 1])

        for co in range(8):
            kproj_chunk(co)
            attn_pair(co)

        psDt.release()
        psDa.release()
        psDs.release()
        rsp.release()
        ptp.release()
        sbD.release()
        mkp.release()
        qkvp.release()
        psC.release()
        wst.release()
        xz.release()
        zTp.release()

        # ---------------- Phase E: LN2 + transpose ---------------------------
        wf1 = tc.alloc_tile_pool(name="wf1", bufs=2, side="right")
        wf2 = tc.alloc_tile_pool(name="wf2", bufs=3, side="right")
        z2Tp = tc.alloc_tile_pool(name="z2T", bufs=1, side="right")
        z2T = z2Tp.tile([P, 8, WINDOW], BF16)
        xz2 = tc.alloc_tile_pool(name="xz2", bufs=3, side="left")
        psE = tc.alloc_tile_pool(name="psE", bufs=3, space="PSUM")

        for t in range(8):
            at = attn[:, t, :]
            st = xz2.tile([P, 20], F32, tag="stats2")
            st6 = st[:, 0:12].rearrange("p (j k) -> p j k", k=6)
            mv = st[:, 12:14]
            veps, sdv, rstd = st[:, 14:15], st[:, 15:16], st[:, 16:17]
            nc.vector.bn_stats(st6[:, 0, :], at[:, 0:512])
            nc.vector.bn_stats(st6[:, 1, :], at[:, 512:1024])
            nc.vector.bn_aggr(mv, st[:, 0:12])
            nc.vector.tensor_scalar(veps, mv[:, 1:2], 1.0, EPS,
                                    op0=ALU.mult, op1=ALU.add)
            nc.scalar.sqrt(sdv, veps)
            nc.vector.reciprocal(rstd, sdv)
            z = xz2.tile([P, D], BF16, tag="zE")
            nc.vector.tensor_scalar(z[:], at, mv[:, 0:1], rstd,
                                    op0=ALU.subtract, op1=ALU.mult)
            for c in range(8):
                tp = psE.tile([P, P], BF16, tag="tpE")
                nc.tensor.transpose(tp[:], z[:, c * P:(c + 1) * P], ident[:])
                nc.scalar.activation(z2T[:, c, t * P:(t + 1) * P], tp[:],
                                     AF.Identity, bias=b2dm[:, c:c + 1],
                                     scale=g2dm[:, c:c + 1])

        psE.release()
        xz2.release()

        # ---------------- Phase F: MLP ---------------------------------------
        # h1 for all of D_FF is computed first (bf16, 64KB/partition), then h2
        # accumulates all 32 contraction chunks in PSUM per (co, qh); the
        # evacuation fuses +b2 and the residual in a single DVE pass.
        xinp = tc.alloc_tile_pool(name="xinp", bufs=4, side="left")
        h1p = tc.alloc_tile_pool(name="h1p", bufs=1, side="left")
        h1all = h1p.tile([P, 32, WINDOW], BF16)
        psF1 = tc.alloc_tile_pool(name="psF1", bufs=4, space="PSUM")
        psF2 = tc.alloc_tile_pool(name="psF2", bufs=4, space="PSUM")
        yp = tc.alloc_tile_pool(name="yp", bufs=2, side="left")

        for sc in range(4):
            w1r = wf1.tile([P, 8, 1024], BF16, tag="w1r")
            for kc in range(8):
                nc.sync.dma_start(
                    w1r[:, kc, :],
                    w1_d[kc * P:(kc + 1) * P, sc * 1024:(sc + 1) * 1024])
            for ft in range(8):
                for qh in range(2):
                    hp = psF1.tile([P, 512], F32, tag="h1ps")
                    for kc in range(8):
                        nc.tensor.matmul(
                            hp[:], w1r[:, kc, ft * P:(ft + 1) * P],
                            z2T[:, kc, qh * 512:(qh + 1) * 512],
                            start=(kc == 0), stop=(kc == 7))
                    nc.scalar.activation(
                        h1all[:, sc * 8 + ft, qh * 512:(qh + 1) * 512],
                        hp[:], AF.Silu,
                        bias=b1s[:, sc * 8 + ft:sc * 8 + ft + 1], scale=1.0)
        z2Tp.release()

        for co in range(8):
            w2r = wf2.tile([P, 32, P], BF16, tag="w2r")
            for fc in range(32):
                nc.sync.dma_start(
                    w2r[:, fc, :],
                    w2_d[fc * P:(fc + 1) * P, co * P:(co + 1) * P])
            xt4 = xinp.tile([P, WINDOW], F32, tag="xinT")
            nc.sync.dma_start(xt4[:], xinT_d[co * P:(co + 1) * P, :])
            y = yp.tile([P, WINDOW], F32, tag="y")
            for qh in range(2):
                hp2 = psF2.tile([P, 512], F32, tag="h2ps")
                for fc in range(32):
                    nc.tensor.matmul(
                        hp2[:], w2r[:, fc, :],
                        h1all[:, fc, qh * 512:(qh + 1) * 512],
                        start=(fc == 0), stop=(fc == 31))
                nc.vector.scalar_tensor_tensor(
                    y[:, qh * 512:(qh + 1) * 512], hp2[:],
                    b2s[:, co:co + 1],
                    xt4[:, qh * 512:(qh + 1) * 512],
                    op0=ALU.add, op1=ALU.add)
            nc.sync.dma_start(y_d[co * P:(co + 1) * P, :], y[:])

        yp.release()
        psF2.release()
        psF1.release()
        h1p.release()
        xinp.release()
        wf2.release()
        wf1.release()
        attnp.release()
        cpool.release()

    nc.compile()
    return nc


def _prep_inputs(inputs):
    x = np.asarray(inputs["x"], dtype=np.float32)
    kpm = np.asarray(inputs["key_pad_mask"]).astype(bool)
    wq = np.asarray(inputs["wq"], dtype=np.float32)
    wkv = np.asarray(inputs["wkv"], dtype=np.float32)
    w1 = np.asarray(inputs["w1"], dtype=np.float32)
    w2 = np.asarray(inputs["w2"], dtype=np.float32)
    bq = np.asarray(inputs["bq"], dtype=np.float32)
    bkv = np.asarray(inputs["bkv"], dtype=np.float32)
    b1 = np.asarray(inputs["b1"], dtype=np.float32)
    b2 = np.asarray(inputs["b2"], dtype=np.float32)
    ln1_g = np.asarray(inputs["ln1_g"], dtype=np.float32)
    ln1_b = np.asarray(inputs["ln1_b"], dtype=np.float32)
    ln2_g = np.asarray(inputs["ln2_g"], dtype=np.float32)
    ln2_b = np.asarray(inputs["ln2_b"], dtype=np.float32)

    def dm(v):  # [D] -> [P, 8] dim-major chunk layout
        return np.ascontiguousarray(v.reshape(8, P).T)

    def bf(a):
        return np.ascontiguousarray(a.astype(ml_dtypes.bfloat16))

    shared = {
        "wq": bf(wq),
        "wkv": bf(wkv),
        "w1": bf(w1),
        "w2": bf(w2),
        "bqs": np.ascontiguousarray((bq * ISD).reshape(8, P).T),
        "bkvk": dm(bkv[0:D]),
        "bkvvb": np.ascontiguousarray(
            np.broadcast_to(bkv[D:2 * D], (P, D)).astype(np.float32)),
        "g1dm": dm(ln1_g),
        "b1dm": dm(ln1_b),
        "g2dm": dm(ln2_g),
        "b2dm": dm(ln2_b),
        "b1s": np.ascontiguousarray(b1.reshape(32, P).T),
        "b2s": dm(b2),
    }

    kwin = np.arange(WINDOW, dtype=np.int32)[:, None]  # window key index
    qloc = np.arange(WINDOW, dtype=np.int32)[None, :]  # local query index
    in_maps = []
    for core in range(8):
        b, h = core // 2, core % 2
        xq = x[b, h * WINDOW:(h + 1) * WINDOW]
        xw = x[b, S - WINDOW:S]
        pad = kpm[b, S - WINDOW:S]  # [WINDOW] per-key pad mask
        biasp = np.where(pad.reshape(8, P).T, np.float32(-160.0),
                         np.float32(0.0))
        keep = (kwin <= h * WINDOW + qloc)  # [WINDOW keys, WINDOW queries]
        mask01 = keep.reshape(8, P, WINDOW).transpose(1, 0, 2)
        m = dict(shared)
        m["xin"] = bf(np.concatenate([xq, xw], axis=0))
        m["xinT"] = np.ascontiguousarray(xq.T)
        m["biasp"] = np.ascontiguousarray(biasp)
        m["mask01"] = bf(mask01.astype(np.float32))
        in_maps.append(m)
    return in_maps


def kernel(**inputs):
    from concourse.bass_utils import run_bass_kernel_spmd

    if "nc" not in _CACHE:
        _CACHE["nc"] = _build_program()
    nc = _CACHE["nc"]

    in_maps = _prep_inputs(inputs)
    trace = os.environ.get("KERNEL_TRACE", "0") == "1"
    res = run_bass_kernel_spmd(nc, in_maps, core_ids=list(range(8)),
                               trace=trace)
    if res.exec_time_ns is not None:
        print(f"HW exec time: {res.exec_time_ns} ns")
        _CACHE["exec_time_ns"] = res.exec_time_ns
    out = np.empty((B, S, D), dtype=np.float32)
    for core in range(8):
        b, h = core // 2, core % 2
        out[b, h * WINDOW:(h + 1) * WINDOW] = res.results[core]["y"].T
    return out


# revision 49
# speedup vs baseline: 1.0924x; 1.0375x over previous
"""Trainium2 Bass kernel for a custom transformer block.

Sharding: 8 cores = 4 batches x 2 sequence halves. Each core computes the
full block (LN1 -> QKV -> windowed attention -> LN2 -> MLP -> residual) for
its 1024 query tokens; the KV window (last 1024 tokens of its batch) is
recomputed on both cores of a batch pair to avoid any collectives.

All matmuls run in bf16 (fp32r costs ~2.2x bf16 per row on HW despite the
cost model claiming parity). Layernorm stats use bn_stats; the pad mask is
folded into the exp bias (per-partition = per-key), and the causal mask is
applied post-exp with tensor_paged_mask using per-core index tables, so no
mask tensor is DMA'd. The MLP accumulates over all of D_FF in PSUM and the
final evacuation fuses +b2 and the residual add in one DVE pass.
"""
import sys
import os

if "/opt/trn_rl_repo" not in sys.path:
    sys.path.insert(0, "/opt/trn_rl_repo")

import numpy as np
import ml_dtypes

B, S, D = 4, 2048, 1024
N_HEAD = 16
D_HEAD = 64
WINDOW = 1024
D_FF = 4096
EPS = 1e-5
ISD = float(1.0 / np.sqrt(D))  # 1/32
P = 128
TINY = 1e-30  # rowsum guard: fully-masked rows -> attn 0 instead of inf/NaN

_CACHE = {}


def _build_program():
    import concourse.bacc as bacc
    import concourse.mybir as mybir
    from concourse.tile import TileContext
    from concourse.masks import make_identity

    F32 = mybir.dt.float32
    BF16 = mybir.dt.bfloat16
    F16 = mybir.dt.float16
    AF = mybir.ActivationFunctionType
    ALU = mybir.AluOpType

    nc = bacc.Bacc("TRN2", target_bir_lowering=False, debug=False,
                   num_devices=8)

    xin_d = nc.dram_tensor("xin", [2 * WINDOW, D], BF16, kind="ExternalInput")
    wq_d = nc.dram_tensor("wq", [D, D], BF16, kind="ExternalInput")
    wkv_d = nc.dram_tensor("wkv", [D, 2 * D], BF16, kind="ExternalInput")
    w1_d = nc.dram_tensor("w1", [D, D_FF], BF16, kind="ExternalInput")
    w2_d = nc.dram_tensor("w2", [D_FF, D], BF16, kind="ExternalInput")
    bqs_d = nc.dram_tensor("bqs", [P, 8], F32, kind="ExternalInput")
    bkvk_d = nc.dram_tensor("bkvk", [P, 8], F32, kind="ExternalInput")
    bkvvb_d = nc.dram_tensor("bkvvb", [P, D], F32, kind="ExternalInput")
    g1dm_d = nc.dram_tensor("g1dm", [P, 8], F32, kind="ExternalInput")
    b1dm_d = nc.dram_tensor("b1dm", [P, 8], F32, kind="ExternalInput")
    g2dm_d = nc.dram_tensor("g2dm", [P, 8], F32, kind="ExternalInput")
    b2dm_d = nc.dram_tensor("b2dm", [P, 8], F32, kind="ExternalInput")
    b1s_d = nc.dram_tensor("b1s", [P, 32], F32, kind="ExternalInput")
    b2s_d = nc.dram_tensor("b2s", [P, 8], F32, kind="ExternalInput")
    biasp_d = nc.dram_tensor("biasp", [P, 8], F32, kind="ExternalInput")
    mask01_d = nc.dram_tensor("mask01", [P, 8, WINDOW], BF16,
                              kind="ExternalInput")
    xinT_d = nc.dram_tensor("xinT", [D, WINDOW], F32, kind="ExternalInput")
    y_d = nc.dram_tensor("y", [D, WINDOW], F32, kind="ExternalOutput")

    with TileContext(nc) as tc:
        cpool = tc.alloc_tile_pool(name="const", bufs=1, side="left")
        ident = cpool.tile([P, P], BF16)
        make_identity(nc, ident[:])
        smallc = cpool.tile([P, 96], F32)
        bqs = smallc[:, 0:8]
        bkvk = smallc[:, 8:16]
        b1s = smallc[:, 16:48]
        b2s = smallc[:, 48:56]
        g1dm = smallc[:, 56:64]
        b1dm = smallc[:, 64:72]
        g2dm = smallc[:, 72:80]
        b2dm = smallc[:, 80:88]
        biasp = smallc[:, 88:96]
        onesc = cpool.tile([P, 16], BF16)
        nc.vector.memset(onesc, 1.0)
        nc.sync.dma_start(bqs, bqs_d[:])
        nc.sync.dma_start(bkvk, bkvk_d[:])
        nc.sync.dma_start(b1s, b1s_d[:])
        nc.sync.dma_start(b2s, b2s_d[:])
        nc.sync.dma_start(g1dm, g1dm_d[:])
        nc.sync.dma_start(b1dm, b1dm_d[:])
        nc.sync.dma_start(g2dm, g2dm_d[:])
        nc.sync.dma_start(b2dm, b2dm_d[:])
        nc.sync.dma_start(biasp, biasp_d[:])

        attnp = tc.alloc_tile_pool(name="attn", bufs=1, side="left")
        attn = attnp.tile([P, 8, D], BF16)         # normalized attn out

        # ---------------- Phase B: LN1 + transpose to dim-major ------------
        zTp = tc.alloc_tile_pool(name="zT", bufs=1, side="left")
        zqT = zTp.tile([P, 8, WINDOW], BF16)
        zwT = zTp.tile([P, 8, WINDOW], BF16)
        xz = tc.alloc_tile_pool(name="xz", bufs=3, side="left")
        psC = tc.alloc_tile_pool(name="psC", bufs=2, space="PSUM")
        psB = tc.alloc_tile_pool(name="psB", bufs=3, space="PSUM")

        def ln1_tile(t):
            xt = xz.tile([P, D], BF16, tag="x")
            nc.sync.dma_start(xt[:], xin_d[t * P:(t + 1) * P, :])
            st = xz.tile([P, 20], F32, tag="stats")
            st6 = st[:, 0:12].rearrange("p (j k) -> p j k", k=6)
            mv = st[:, 12:14]
            veps, sdv, rstd = st[:, 14:15], st[:, 15:16], st[:, 16:17]
            nc.vector.bn_stats(st6[:, 0, :], xt[:, 0:512])
            nc.vector.bn_stats(st6[:, 1, :], xt[:, 512:1024])
            nc.vector.bn_aggr(mv, st[:, 0:12])
            nc.vector.tensor_scalar(veps, mv[:, 1:2], 1.0, EPS,
                                    op0=ALU.mult, op1=ALU.add)
            nc.scalar.sqrt(sdv, veps)
            nc.vector.reciprocal(rstd, sdv)
            z = xz.tile([P, D], BF16, tag="z")
            nc.vector.tensor_scalar(z[:], xt[:], mv[:, 0:1], rstd,
                                    op0=ALU.subtract, op1=ALU.mult)
            dst = zqT if t < 8 else zwT
            col = (t % 8) * P
            for c in range(8):
                tp = psB.tile([P, P], BF16, tag="tpB")
                nc.tensor.transpose(tp[:], z[:, c * P:(c + 1) * P], ident[:])
                nc.scalar.activation(dst[:, c, col:col + P], tp[:],
                                     AF.Identity, bias=b1dm[:, c:c + 1],
                                     scale=g1dm[:, c:c + 1])

        for t in range(8):
            ln1_tile(t)

        qkvp = tc.alloc_tile_pool(name="qkv", bufs=1, side="right")
        qT = qkvp.tile([P, 8, WINDOW], BF16)      # q/sqrt(D), dim-major
        kT = qkvp.tile([P, 8, WINDOW], BF16)      # k, dim-major
        V = qkvp.tile([P, 8, N_HEAD * 65], BF16)  # token-major + ones col

        wst = tc.alloc_tile_pool(name="wst", bufs=2, side="left")

        # Q: weights stationary -> qT dim-major, scaled by 1/32
        for wh in range(2):
            wqr = wst.tile([P, 8, 512], BF16, tag="wkres")
            nc.sync.dma_start(
                wqr[:],
                wq_d[:, wh * 512:(wh + 1) * 512].rearrange(
                    "(k p) n -> p k n", p=P))
            for co in range(wh * 4, wh * 4 + 4):
                for qh in range(2):
                    pp = psC.tile([P, 512], F32, tag="proj")
                    for kc in range(8):
                        nc.tensor.matmul(
                            pp[:], wqr[:, kc, (co % 4) * P:(co % 4 + 1) * P],
                            zqT[:, kc, qh * 512:(qh + 1) * 512],
                            start=(kc == 0), stop=(kc == 7))
                    nc.scalar.activation(
                        qT[:, co, qh * 512:(qh + 1) * 512], pp[:],
                        AF.Identity, bias=bqs[:, co:co + 1], scale=ISD)

        for t in range(8, 16):
            ln1_tile(t)
        psB.release()

        # V: activations stationary -> token-major, bias added via bcast tile
        bkvvb = wst.tile([P, D], F32, tag="bkvvb")
        nc.sync.dma_start(bkvvb[:], bkvvb_d[:])
        for vh in range(2):
            wvr = wst.tile([P, 8, 512], BF16, tag="wkres")
            nc.sync.dma_start(
                wvr[:],
                wkv_d[:, D + vh * 512:D + (vh + 1) * 512].rearrange(
                    "(k p) n -> p k n", p=P))
            for tt in range(8):
                pp = psC.tile([P, 512], F32, tag="proj")
                for kc in range(8):
                    nc.tensor.matmul(
                        pp[:], zwT[:, kc, tt * P:(tt + 1) * P],
                        wvr[:, kc, :],
                        start=(kc == 0), stop=(kc == 7))
                vdst = V[:, tt, :].rearrange("p (h n) -> p h n", n=65)[
                    :, vh * 8:(vh + 1) * 8, 0:64]
                nc.vector.scalar_tensor_tensor(
                    vdst, pp[:].rearrange("p (h n) -> p h n", n=64), 0.0,
                    bkvvb[:, vh * 512:(vh + 1) * 512].rearrange(
                        "p (h n) -> p h n", n=64),
                    op0=ALU.add, op1=ALU.add)
        for tt in range(8):
            nc.scalar.copy(
                V[:, tt, :].rearrange("p (h n) -> p h n", n=65)[:, :, 64:65],
                onesc.rearrange("p (h n) -> p h n", n=1))

        # ---------------- Phase D: K proj interleaved with attention -------
        # K chunk co covers dims [co*128, (co+1)*128) = heads 2*co, 2*co+1,
        # so attention head-pair co can start right after K chunk co.
        mkp = tc.alloc_tile_pool(name="mk", bufs=1, side="left")
        mask01 = mkp.tile([P, 8, WINDOW], BF16)
        nc.sync.dma_start(mask01[:], mask01_d[:])
        sbD = tc.alloc_tile_pool(name="sbD", bufs=2, side="left")
        ptp = tc.alloc_tile_pool(name="ptp", bufs=20, side="right")
        rsp = tc.alloc_tile_pool(name="rsp", bufs=4, side="left")
        psDs = tc.alloc_tile_pool(name="psDs", bufs=4, space="PSUM")
        psDa = tc.alloc_tile_pool(name="psDa", bufs=1, space="PSUM")
        psDt = tc.alloc_tile_pool(name="psDt", bufs=1, space="PSUM")

        wkk = [None, None]

        def kproj_chunk(co):
            wh, cq = co // 4, co % 4
            if cq == 0:
                wkr = wst.tile([P, 8, 512], BF16, tag="wkres")
                wkk[wh] = wkr
                nc.sync.dma_start(
                    wkr[:],
                    wkv_d[:, wh * 512:(wh + 1) * 512].rearrange(
                        "(k p) n -> p k n", p=P))
            wkr = wkk[wh]
            for qh in range(2):
                pp = psC.tile([P, 512], F32, tag="proj")
                for kc in range(8):
                    nc.tensor.matmul(
                        pp[:], wkr[:, kc, cq * P:(cq + 1) * P],
                        zwT[:, kc, qh * 512:(qh + 1) * 512],
                        start=(kc == 0), stop=(kc == 7))
                nc.vector.tensor_scalar(
                    kT[:, co, qh * 512:(qh + 1) * 512], pp[:],
                    bkvk[:, co:co + 1], None, op0=ALU.add)

        def attn_pair(hp):
            pair = (2 * hp, 2 * hp + 1)
            # scores + exp + causal mask; consecutive matmuls alternate PE
            # row groups (partitions 0-63 vs 64-127) so LDWEIGHTS pulls ahead
            pts = {h: [] for h in pair}
            for kc in range(8):
                pss = {}
                for qh in range(2):
                    for h in pair:
                        po, ch = (h % 2) * 64, h // 2
                        stile = psDs.tile([P, 512], F32, tag="s")
                        pss[(h, qh)] = stile
                        nc.tensor.matmul(
                            stile[:],
                            kT[po:po + 64, ch, kc * P:(kc + 1) * P],
                            qT[po:po + 64, ch, qh * 512:(qh + 1) * 512],
                            start=True, stop=True)
                for h in pair:
                    ptile = ptp.tile([P, 1024], BF16, tag="pt")
                    pts[h].append(ptile)
                    # exp with pad mask folded into the per-key bias
                    for qh in range(2):
                        nc.scalar.activation(
                            ptile[:, qh * 512:(qh + 1) * 512],
                            pss[(h, qh)][:], AF.Exp,
                            bias=biasp[:, kc:kc + 1])
                    # causal mask: multiply probs by 0/1 (kc<4, qh=1 is
                    # never causally masked on any core -> skip that half)
                    ncol = 1024 if kc >= 4 else 512
                    nc.vector.tensor_tensor(
                        ptile[:, 0:ncol], ptile[:, 0:ncol],
                        mask01[:, kc, 0:ncol], op=ALU.mult)
            for h in pair:
                oa = sbD.tile([65, 1024], BF16, tag="oa")
                for qh in range(2):
                    avp = psDa.tile([65, 512], F32, tag="av")
                    for kc in range(8):
                        nc.tensor.matmul(
                            avp[:], V[:, kc, h * 65:(h + 1) * 65],
                            pts[h][kc][:, qh * 512:(qh + 1) * 512],
                            start=(kc == 0), stop=(kc == 7))
                    nc.vector.tensor_copy(oa[:, qh * 512:(qh + 1) * 512],
                                          avp[:])
                for tq in range(2):
                    tp4 = psDt.tile([P, 4, 66], BF16, tag="tp4")
                    for j in range(4):
                        t = tq * 4 + j
                        nc.tensor.transpose(tp4[:, j, 0:65],
                                            oa[:, t * P:(t + 1) * P],
                                            ident[0:65, 0:65])
                    rs = rsp.tile([P, 8], F32, tag="rs")
                    rs4, rinv4 = rs[:, 0:4], rs[:, 4:8]
                    nc.vector.tensor_scalar(rs4, tp4[:, :, 64:65], TINY, None,
                                            op0=ALU.add)
                    nc.vector.reciprocal(rinv4, rs4)
                    for j in range(4):
                        t = tq * 4 + j
                        nc.vector.tensor_scalar_mul(
                            attn[:, t, h * 64:(h + 1) * 64],
                            tp4[:, j, 0:64], rinv4[:, j:j + 1])

        for co in range(8):
            kproj_chunk(co)
            attn_pair(co)

        psDt.release()
        psDa.release()
        psDs.release()
        rsp.release()
        ptp.release()
        sbD.release()
        mkp.release()
        qkvp.release()
        psC.release()
        wst.release()
        xz.release()
        zTp.release()

        # ---------------- Phase E: LN2 + transpose ---------------------------
        wf1 = tc.alloc_tile_pool(name="wf1", bufs=2, side="right")
        wf2 = tc.alloc_tile_pool(name="wf2", bufs=3, side="right")
        z2Tp = tc.alloc_tile_pool(name="z2T", bufs=1, side="right")
        z2T = z2Tp.tile([P, 8, WINDOW], BF16)
        xz2 = tc.alloc_tile_pool(name="xz2", bufs=3, side="left")
        psE = tc.alloc_tile_pool(name="psE", bufs=3, space="PSUM")

        for t in range(8):
            at = attn[:, t, :]
            st = xz2.tile([P, 20], F32, tag="stats2")
            st6 = st[:, 0:12].rearrange("p (j k) -> p j k", k=6)
            mv = st[:, 12:14]
            veps, sdv, rstd = st[:, 14:15], st[:, 15:16], st[:, 16:17]
            nc.vector.bn_stats(st6[:, 0, :], at[:, 0:512])
            nc.vector.bn_stats(st6[:, 1, :], at[:, 512:1024])
            nc.vector.bn_aggr(mv, st[:, 0:12])
            nc.vector.tensor_scalar(veps, mv[:, 1:2], 1.0, EPS,
                                    op0=ALU.mult, op1=ALU.add)
            nc.scalar.sqrt(sdv, veps)
            nc.vector.reciprocal(rstd, sdv)
            z = xz2.tile([P, D], BF16, tag="zE")
            nc.vector.tensor_scalar(z[:], at, mv[:, 0:1], rstd,
                                    op0=ALU.subtract, op1=ALU.mult)
            for c in range(8):
                tp = psE.tile([P, P], BF16, tag="tpE")
                nc.tensor.transpose(tp[:], z[:, c * P:(c + 1) * P], ident[:])
                nc.vector.tensor_scalar(z2T[:, c, t * P:(t + 1) * P], tp[:],
                                        g2dm[:, c:c + 1], b2dm[:, c:c + 1],
                                        op0=ALU.mult, op1=ALU.add)

        psE.release()
        xz2.release()

        # ---------------- Phase F: MLP ---------------------------------------
        # h1 for all of D_FF is computed first (bf16, 64KB/partition), then h2
        # accumulates all 32 contraction chunks in PSUM per (co, qh); the
        # evacuation fuses +b2 and the residual in a single DVE pass.
        xinp = tc.alloc_tile_pool(name="xinp", bufs=4, side="left")
        h1p = tc.alloc_tile_pool(name="h1p", bufs=1, side="left")
        h1all = h1p.tile([P, 32, WINDOW], BF16)
        psF1 = tc.alloc_tile_pool(name="psF1", bufs=4, space="PSUM")
        psF2 = tc.alloc_tile_pool(name="psF2", bufs=4, space="PSUM")
        yp = tc.alloc_tile_pool(name="yp", bufs=2, side="left")

        for sc in range(4):
            w1r = wf1.tile([P, 8, 1024], BF16, tag="w1r")
            nc.sync.dma_start(
                w1r[:],
                w1_d[:, sc * 1024:(sc + 1) * 1024].rearrange(
                    "(k p) n -> p k n", p=P))
            for ft in range(8):
                for qh in range(2):
                    hp = psF1.tile([P, 512], F32, tag="h1ps")
                    for kc in range(8):
                        nc.tensor.matmul(
                            hp[:], w1r[:, kc, ft * P:(ft + 1) * P],
                            z2T[:, kc, qh * 512:(qh + 1) * 512],
                            start=(kc == 0), stop=(kc == 7))
                    nc.scalar.activation(
                        h1all[:, sc * 8 + ft, qh * 512:(qh + 1) * 512],
                        hp[:], AF.Silu,
                        bias=b1s[:, sc * 8 + ft:sc * 8 + ft + 1], scale=1.0)
        z2Tp.release()

        for co in range(8):
            w2r = wf2.tile([P, 32, P], BF16, tag="w2r")
            nc.sync.dma_start(
                w2r[:],
                w2_d[:, co * P:(co + 1) * P].rearrange(
                    "(f p) n -> p f n", p=P))
            xt4 = xinp.tile([P, WINDOW], F32, tag="xinT")
            nc.sync.dma_start(xt4[:], xinT_d[co * P:(co + 1) * P, :])
            y = yp.tile([P, WINDOW], F32, tag="y")
            for qh in range(2):
                hp2 = psF2.tile([P, 512], F32, tag="h2ps")
                for fc in range(32):
                    nc.tensor.matmul(
                        hp2[:], w2r[:, fc, :],
                        h1all[:, fc, qh * 512:(qh + 1) * 512],
                        start=(fc == 0), stop=(fc == 31))
                nc.vector.scalar_tensor_tensor(
                    y[:, qh * 512:(qh + 1) * 512], hp2[:],
                    b2s[:, co:co + 1],
                    xt4[:, qh * 512:(qh + 1) * 512],
                    op0=ALU.add, op1=ALU.add)
            nc.sync.dma_start(y_d[co * P:(co + 1) * P, :], y[:])

        yp.release()
        psF2.release()
        psF1.release()
        h1p.release()
        xinp.release()
        wf2.release()
        wf1.release()
        attnp.release()
        cpool.release()

    nc.compile()
    return nc


def _prep_inputs(inputs):
    x = np.asarray(inputs["x"], dtype=np.float32)
    kpm = np.asarray(inputs["key_pad_mask"]).astype(bool)
    wq = np.asarray(inputs["wq"], dtype=np.float32)
    wkv = np.asarray(inputs["wkv"], dtype=np.float32)
    w1 = np.asarray(inputs["w1"], dtype=np.float32)
    w2 = np.asarray(inputs["w2"], dtype=np.float32)
    bq = np.asarray(inputs["bq"], dtype=np.float32)
    bkv = np.asarray(inputs["bkv"], dtype=np.float32)
    b1 = np.asarray(inputs["b1"], dtype=np.float32)
    b2 = np.asarray(inputs["b2"], dtype=np.float32)
    ln1_g = np.asarray(inputs["ln1_g"], dtype=np.float32)
    ln1_b = np.asarray(inputs["ln1_b"], dtype=np.float32)
    ln2_g = np.asarray(inputs["ln2_g"], dtype=np.float32)
    ln2_b = np.asarray(inputs["ln2_b"], dtype=np.float32)

    def dm(v):  # [D] -> [P, 8] dim-major chunk layout
        return np.ascontiguousarray(v.reshape(8, P).T)

    def bf(a):
        return np.ascontiguousarray(a.astype(ml_dtypes.bfloat16))

    shared = {
        "wq": bf(wq),
        "wkv": bf(wkv),
        "w1": bf(w1),
        "w2": bf(w2),
        "bqs": np.ascontiguousarray((bq * ISD).reshape(8, P).T),
        "bkvk": dm(bkv[0:D]),
        "bkvvb": np.ascontiguousarray(
            np.broadcast_to(bkv[D:2 * D], (P, D)).astype(np.float32)),
        "g1dm": dm(ln1_g),
        "b1dm": dm(ln1_b),
        "g2dm": dm(ln2_g),
        "b2dm": dm(ln2_b),
        "b1s": np.ascontiguousarray(b1.reshape(32, P).T),
        "b2s": dm(b2),
    }

    kwin = np.arange(WINDOW, dtype=np.int32)[:, None]  # window key index
    qloc = np.arange(WINDOW, dtype=np.int32)[None, :]  # local query index
    in_maps = []
    for core in range(8):
        b, h = core // 2, core % 2
        xq = x[b, h * WINDOW:(h + 1) * WINDOW]
        xw = x[b, S - WINDOW:S]
        pad = kpm[b, S - WINDOW:S]  # [WINDOW] per-key pad mask
        biasp = np.where(pad.reshape(8, P).T, np.float32(-160.0),
                         np.float32(0.0))
        keep = (kwin <= h * WINDOW + qloc)  # [WINDOW keys, WINDOW queries]
        mask01 = keep.reshape(8, P, WINDOW).transpose(1, 0, 2)
        m = dict(shared)
        m["xin"] = bf(np.concatenate([xq, xw], axis=0))
        m["xinT"] = np.ascontiguousarray(xq.T)
        m["biasp"] = np.ascontiguousarray(biasp)
        m["mask01"] = bf(mask01.astype(np.float32))
        in_maps.append(m)
    return in_maps


def kernel(**inputs):
    from concourse.bass_utils import run_bass_kernel_spmd

    if "nc" not in _CACHE:
        _CACHE["nc"] = _build_program()
    nc = _CACHE["nc"]

    in_maps = _prep_inputs(inputs)
    trace = os.environ.get("KERNEL_TRACE", "0") == "1"
    res = run_bass_kernel_spmd(nc, in_maps, core_ids=list(range(8)),
                               trace=trace)
    if res.exec_time_ns is not None:
        print(f"HW exec time: {res.exec_time_ns} ns")
        _CACHE["exec_time_ns"] = res.exec_time_ns
    out = np.empty((B, S, D), dtype=np.float32)
    for core in range(8):
        b, h = core // 2, core % 2
        out[b, h * WINDOW:(h + 1) * WINDOW] = res.results[core]["y"].T
    return out
